# revision 1
# baseline (speedup 1.0000x reference)
"""Trainium2 Bass kernel for dense transformer block nn_Block_68221260529679.

Layout: B=2, T=2048, D=2048, N=8 q-heads, K=1 kv-head, H=256, F=16384.

Sharding (8 NeuronCores): DP over batch (2 groups of 4) x TP within group (4).
Core c = 4*b + r: batch b, q-heads {HEADS*r .. HEADS*(r+1)}, MLP hidden rows
[r*FS, (r+1)*FS).  Within each 4-core group:
  - every core computes the full rmsnorm(x) and the shared k/v projection
    (K=1 kv-head) redundantly,
  - attention + o-proj partial summed over the core's heads -> ReduceScatter
    (each core keeps T-slice r),
  - x2 = x + attn on the slice, rmsnorm, transpose -> AllGather of h2^T,
  - gate/up/gelu/down on the F-shard over all T -> ReduceScatter,
  - out slice = x2 + down.  Host assembles the 8 [T/4, D] slices.

All matmuls in bf16 with fp32 PSUM accumulation; norms/softmax/residuals fp32.
The rmsnorm scales (1+scale) and the q scaling H^-0.5 are folded into the
weights host-side; rope sin/cos tables and the additive mask bias are
precomputed host-side per batch.  Weights are pre-laid-out host-side so every
DMA moves contiguous >=8KB lines per partition.
"""

from contextlib import ExitStack

import numpy as np
import ml_dtypes

import concourse.bass as bass
import concourse.mybir as mybir
import concourse.tile as tile
from concourse import bacc
from concourse.masks import make_identity

F32 = mybir.dt.float32
BF16 = mybir.dt.bfloat16
AF = mybir.ActivationFunctionType
ALU = mybir.AluOpType
BIG_NEG = -2.3819763e38
GROUPS = [[0, 1, 2, 3], [4, 5, 6, 7]]

FULL_CFG = dict(T=2048, D=2048, H=256, HEADS=2, FS=4096)


def build(cfg):
    T, D, H, HEADS, FS = cfg["T"], cfg["D"], cfg["H"], cfg["HEADS"], cfg["FS"]
    REPS = cfg.get("reps", 1)
    assert H == 256
    TP = 4
    TT, DT, FB = T // 128, D // 128, FS // 128
    TCH = T // TP              # attention/MLP T-chunk == rank slice
    NCH, CHT, DCH = T // TCH, TCH // 128, D // TCH
    SLICE = TCH
    ST = SLICE // 128
    BS = TCH // TP             # per-rank row block within a T-chunk
    QB = HEADS * H // 128      # q col blocks (2 per head)

    nc = bacc.Bacc("TRN2", target_bir_lowering=False, debug=False, num_devices=8)
    x_ext = nc.dram_tensor("x", [T, D], F32, kind="ExternalInput").ap()
    xs_ext = nc.dram_tensor("x_slice", [SLICE, D], F32, kind="ExternalInput").ap()
    wq_ext = nc.dram_tensor("wq", [128, DT, HEADS * H], BF16,
                            kind="ExternalInput").ap()
    wkv_ext = nc.dram_tensor("wkv", [128, DT, 2 * H], BF16,
                             kind="ExternalInput").ap()
    wo_ext = nc.dram_tensor("wo", [128, QB, D], BF16, kind="ExternalInput").ap()
    wg_ext = nc.dram_tensor("wg", [FB, 128, DT, 256], BF16,
                            kind="ExternalInput").ap()
    wl_ext = nc.dram_tensor("wl", [DCH, 128, FB, TCH], BF16,
                            kind="ExternalInput").ap()
    sin_ext = nc.dram_tensor("sin", [H // 2, T], F32, kind="ExternalInput").ap()
    cos_ext = nc.dram_tensor("cos", [H // 2, T], F32, kind="ExternalInput").ap()
    out_ext = nc.dram_tensor("out", [SLICE, D], F32, kind="ExternalOutput").ap()

    with tile.TileContext(nc) as tc, ExitStack() as top:
        cons = top.enter_context(tc.tile_pool(name="cons", bufs=1))
        dram = top.enter_context(tc.tile_pool(name="dram", bufs=1, space="DRAM"))

        ident = cons.tile([128, 128], BF16)
        make_identity(nc, ident)
        eps = cons.tile([128, 1], F32)
        nc.vector.memset(eps, 1e-6)
        sin_sb = cons.tile([128, T], F32)
        nc.sync.dma_start(out=sin_sb, in_=sin_ext)
        cos_sb = cons.tile([128, T], F32)
        nc.sync.dma_start(out=cos_sb, in_=cos_ext)

        # DRAM intermediates, one set per T-chunk (chunked collectives)
        attn_d = [dram.tile([TCH, D], BF16, tag=f"attn_d{i}", name=f"attn_d{i}") for i in range(NCH)]
        attn_r = [dram.tile([BS, D], BF16, tag=f"attn_r{i}", name=f"attn_r{i}") for i in range(NCH)]
        h2o = [dram.tile([D, BS], BF16, tag=f"h2o{i}", name=f"h2o{i}") for i in range(NCH)]
        h2g = [dram.tile([TP * D, BS], BF16, tag=f"h2g{i}", name=f"h2g{i}") for i in range(NCH)]
        down_d = [dram.tile([TCH, D], BF16, tag=f"down_d{i}", name=f"down_d{i}") for i in range(NCH)]
        down_r = [dram.tile([BS, D], BF16, tag=f"down_r{i}", name=f"down_r{i}") for i in range(NCH)]
        x2_d = [dram.tile([BS, D], F32, tag=f"x2_d{i}", name=f"x2_d{i}") for i in range(NCH)]

        for _rep in range(REPS):
            with ExitStack() as attn_sc:
                acts = attn_sc.enter_context(tc.tile_pool(name="acts", bufs=1))
                qT = acts.tile([128, 2 * HEADS, T], BF16, tag="qT")
                kT = acts.tile([128, 2, T], BF16, tag="kT")
                v_sb = acts.tile([128, TT, H], BF16, tag="v")

                # ---------------- P1: rmsnorm(x) + transpose -> hT --------
                with (
                    tc.tile_pool(name="p12", bufs=2) as p12,
                    tc.tile_pool(name="ps12", bufs=2, space="PSUM") as ps12,
                ):
                    hT = p12.tile([128, DT, T], BF16, tag="hT", bufs=1)
                    for t in range(TT):
                        tsl = slice(t * 128, (t + 1) * 128)
                        xt = p12.tile([128, D], F32, tag="xt")
                        nc.sync.dma_start(out=xt, in_=x_ext[tsl])
                        h = p12.tile([128, D], BF16, tag="h")
                        ssq = p12.tile([128, 1], F32, tag="ssq")
                        nc.scalar.activation(out=h, in_=xt, func=AF.Square,
                                             accum_out=ssq)
                        rstd = p12.tile([128, 1], F32, tag="rstd")
                        nc.scalar.activation(out=rstd, in_=ssq, func=AF.Sqrt,
                                             bias=eps, scale=1.0 / D)
                        nc.vector.reciprocal(out=rstd, in_=rstd)
                        nc.vector.tensor_scalar_mul(h, xt, rstd)
                        for kd in range(DT):
                            pt = ps12.tile([128, 128], BF16, tag="tp")
                            nc.tensor.transpose(
                                pt, h[:, kd * 128:(kd + 1) * 128], ident)
                            nc.vector.tensor_copy(hT[:, kd, tsl], pt)

                    # ------------ P2: q/k/v projections + rope ------------
                    wqs = p12.tile([128, DT, HEADS * H], BF16, tag="wqs",
                                   bufs=1)
                    nc.sync.dma_start(out=wqs, in_=wq_ext)
                    wkvs = p12.tile([128, DT, 2 * H], BF16, tag="wkvs", bufs=1)
                    nc.sync.dma_start(out=wkvs, in_=wkv_ext)

                    def rope_pair(dst, blk1, blk2, x1p, x2p, csl):
                        cs, sn = cos_sb[:, csl], sin_sb[:, csl]
                        t1 = p12.tile([128, TCH], F32, tag="rp1")
                        t2 = p12.tile([128, TCH], F32, tag="rp2")
                        nc.vector.tensor_tensor(t1, x1p, cs, op=ALU.mult)
                        nc.vector.tensor_tensor(t2, x2p, sn, op=ALU.mult)
                        nc.vector.tensor_tensor(dst[:, blk1, csl], t1, t2,
                                                op=ALU.subtract)
                        nc.vector.tensor_tensor(t1, x2p, cs, op=ALU.mult)
                        nc.vector.tensor_tensor(t2, x1p, sn, op=ALU.mult)
                        nc.vector.tensor_tensor(dst[:, blk2, csl], t1, t2,
                                                op=ALU.add)

                    for ch in range(NCH):
                        csl = slice(ch * TCH, (ch + 1) * TCH)
                        for hd in range(HEADS):   # q heads
                            p1 = ps12.tile([128, TCH], F32, tag="qk1")
                            p2 = ps12.tile([128, TCH], F32, tag="qk2")
                            for kd in range(DT):
                                c0 = (2 * hd) * 128
                                nc.tensor.matmul(p1, wqs[:, kd, c0:c0 + 128],
                                                 hT[:, kd, csl],
                                                 start=kd == 0,
                                                 stop=kd == DT - 1)
                            for kd in range(DT):
                                c1 = (2 * hd + 1) * 128
                                nc.tensor.matmul(p2, wqs[:, kd, c1:c1 + 128],
                                                 hT[:, kd, csl],
                                                 start=kd == 0,
                                                 stop=kd == DT - 1)
                            rope_pair(qT, 2 * hd, 2 * hd + 1, p1, p2, csl)
                        # k
                        p1 = ps12.tile([128, TCH], F32, tag="qk1")
                        p2 = ps12.tile([128, TCH], F32, tag="qk2")
                        for kd in range(DT):
                            nc.tensor.matmul(p1, wkvs[:, kd, 0:128],
                                             hT[:, kd, csl],
                                             start=kd == 0, stop=kd == DT - 1)
                        for kd in range(DT):
                            nc.tensor.matmul(p2, wkvs[:, kd, 128:256],
                                             hT[:, kd, csl],
                                             start=kd == 0, stop=kd == DT - 1)
                        rope_pair(kT, 0, 1, p1, p2, csl)
                        # v (natural layout [S, H])
                        for st in range(ch * CHT, (ch + 1) * CHT):
                            pv = ps12.tile([128, H], F32, tag="vv")
                            for kd in range(DT):
                                nc.tensor.matmul(
                                    pv, hT[:, kd, st * 128:(st + 1) * 128],
                                    wkvs[:, kd, H:2 * H],
                                    start=kd == 0, stop=kd == DT - 1)
                            nc.vector.tensor_copy(v_sb[:, st], pv)

                # ---------------- P3: attention + o-proj ----------------
                with (
                    tc.tile_pool(name="p3", bufs=2) as p3,
                    tc.tile_pool(name="ps3", bufs=2, space="PSUM") as ps3,
                ):
                    wos = p3.tile([128, QB, D], BF16, tag="wos", bufs=1)
                    nc.sync.dma_start(out=wos, in_=wo_ext)

                    def p4_block(ch):
                        xt = p3.tile([BS, D], F32, tag="xs")
                        nc.sync.dma_start(out=xt,
                                          in_=xs_ext[ch * BS:(ch + 1) * BS])
                        ar = p3.tile([BS, D], BF16, tag="ar")
                        nc.sync.dma_start(out=ar, in_=attn_r[ch])
                        x2t = p3.tile([BS, D], F32, tag="x2t")
                        nc.vector.tensor_tensor(x2t, xt, ar, op=ALU.add)
                        nc.sync.dma_start(out=x2_d[ch], in_=x2t)
                        h2 = p3.tile([BS, D], BF16, tag="h2")
                        ssq = p3.tile([BS, 1], F32, tag="ssq2")
                        nc.scalar.activation(out=h2, in_=x2t, func=AF.Square,
                                             accum_out=ssq)
                        rstd = p3.tile([BS, 1], F32, tag="rstd2")
                        nc.scalar.activation(out=rstd, in_=ssq, func=AF.Sqrt,
                                             bias=eps[:BS], scale=1.0 / D)
                        nc.vector.reciprocal(out=rstd, in_=rstd)
                        nc.vector.tensor_scalar_mul(h2, x2t, rstd)
                        h2tb = p3.tile([128, DT, BS], BF16, tag="h2tb")
                        for kd in range(DT):
                            pt = ps3.tile([128, BS], BF16, tag="tp")
                            nc.tensor.transpose(
                                pt, h2[:, kd * 128:(kd + 1) * 128],
                                ident[:BS, :BS])
                            nc.vector.tensor_copy(h2tb[:, kd], pt)
                        nc.sync.dma_start(
                            out=h2o[ch].rearrange("(a p) s -> p a s", p=128),
                            in_=h2tb)
                        nc.gpsimd.collective_compute(
                            "AllGather", ALU.bypass, replica_groups=GROUPS,
                            ins=[h2o[ch].opt()], outs=[h2g[ch].opt()])

                    for ch in range(NCH):
                        encT = p3.tile([128, HEADS, 2, TCH], BF16, tag="encT")
                        for hd in range(HEADS):
                            nsc = ch + 1            # causal S-chunks
                            nS = nsc * CHT          # causal S-tiles
                            pT = p3.tile([128, nS, TCH], BF16, tag="pT",
                                         bufs=1)
                            for tt in range(CHT):
                                gt = ch * CHT + tt
                                gsl = slice(gt * 128, (gt + 1) * 128)
                                seff = nsc * TCH
                                mb = p3.tile([128, T], BF16, tag="mb")
                                nc.gpsimd.memset(mb[:, :seff], 0.0)
                                nc.gpsimd.affine_select(
                                    out=mb[:, :seff], in_=mb[:, :seff],
                                    compare_op=ALU.is_ge, fill=BIG_NEG,
                                    base=gt * 128, channel_multiplier=1,
                                    pattern=[[-1, seff]])
                                lg = p3.tile([128, T], F32, tag="lg")
                                for sc in range(nsc):
                                    ssl = slice(sc * TCH, (sc + 1) * TCH)
                                    pl = ps3.tile([128, TCH], F32, tag="lgp")
                                    nc.tensor.matmul(pl, qT[:, 2 * hd, gsl],
                                                     kT[:, 0, ssl],
                                                     start=True, stop=False)
                                    nc.tensor.matmul(pl,
                                                     qT[:, 2 * hd + 1, gsl],
                                                     kT[:, 1, ssl],
                                                     start=False, stop=True)
                                    nc.vector.tensor_tensor(
                                        lg[:, ssl], pl, mb[:, ssl], op=ALU.add)
                                nmax = p3.tile([128, 1], F32, tag="nmax")
                                nc.vector.tensor_reduce(
                                    nmax, lg[:, :seff],
                                    axis=mybir.AxisListType.X,
                                    op=ALU.max, negate=True)
                                sume = p3.tile([128, 1], F32, tag="sume")
                                nc.scalar.activation(
                                    out=lg[:, :seff], in_=lg[:, :seff],
                                    func=AF.Exp, bias=nmax, accum_out=sume)
                                rsum = p3.tile([128, 1], F32, tag="rsum")
                                nc.vector.reciprocal(rsum, sume)
                                pb = p3.tile([128, T], BF16, tag="pbf")
                                nc.vector.tensor_scalar_mul(
                                    pb[:, :seff], lg[:, :seff], rsum)
                                for s in range(nS):
                                    ptp = ps3.tile([128, 128], BF16, tag="tp")
                                    nc.tensor.transpose(
                                        ptp, pb[:, s * 128:(s + 1) * 128],
                                        ident)
                                    nc.vector.tensor_copy(
                                        pT[:, s, tt * 128:(tt + 1) * 128], ptp)
                            for m in range(2):
                                pe_ = ps3.tile([128, TCH], F32, tag="enc")
                                for s in range(nS):
                                    nc.tensor.matmul(
                                        pe_,
                                        v_sb[:, s, m * 128:(m + 1) * 128],
                                        pT[:, s, :],
                                        start=s == 0, stop=s == nS - 1)
                                nc.vector.tensor_copy(encT[:, hd, m], pe_)
                        # o-proj for this chunk, summed over the core's heads
                        for tt in range(CHT):
                            gt = ch * CHT + tt
                            ao = p3.tile([128, D], BF16, tag="ao")
                            for dch in range(DCH):
                                dsl = slice(dch * TCH, (dch + 1) * TCH)
                                po = ps3.tile([128, TCH], F32, tag="oproj")
                                kk = 0
                                for hd in range(HEADS):
                                    for m in range(2):
                                        nc.tensor.matmul(
                                            po,
                                            encT[:, hd, m,
                                                 tt * 128:(tt + 1) * 128],
                                            wos[:, 2 * hd + m, dsl],
                                            start=kk == 0,
                                            stop=kk == 2 * HEADS - 1)
                                        kk += 1
                                nc.vector.tensor_copy(ao[:, dsl], po)
                            nc.sync.dma_start(
                                out=attn_d[ch][tt * 128:(tt + 1) * 128],
                                in_=ao)
                        # issue the chunk RS now; the residual/norm/AG for
                        # this chunk is emitted during the NEXT chunk's
                        # attention so the in-order engines don't stall on
                        # the collective.
                        nc.gpsimd.collective_compute(
                            "ReduceScatter", ALU.add, replica_groups=GROUPS,
                            ins=[attn_d[ch].opt()], outs=[attn_r[ch].opt()])
                        if ch > 0:
                            p4_block(ch - 1)
                    p4_block(NCH - 1)

            # ------------ P5: MLP on F-shard over all T ------------
            with (
                tc.tile_pool(name="p5", bufs=2) as p5,
                tc.tile_pool(name="ps5", bufs=2, space="PSUM") as ps5,
            ):
                def final_block(r):
                    dr = p5.tile([BS, D], BF16, tag="dr", bufs=1)
                    nc.sync.dma_start(out=dr, in_=down_r[r])
                    x2f = p5.tile([BS, D], F32, tag="x2f", bufs=1)
                    nc.sync.dma_start(out=x2f, in_=x2_d[r])
                    ot = p5.tile([BS, D], F32, tag="ot", bufs=1)
                    nc.vector.tensor_tensor(ot, x2f, dr, op=ALU.add)
                    nc.sync.dma_start(out=out_ext[r * BS:(r + 1) * BS],
                                      in_=ot)

                for r in range(NCH):
                    h2c = p5.tile([128, DT, TCH], BF16, tag="h2c")
                    for j in range(TP):
                        nc.sync.dma_start(
                            out=h2c[:, :, j * BS:(j + 1) * BS],
                            in_=h2g[r][j * D:(j + 1) * D].rearrange(
                                "(a p) s -> p a s", p=128))
                    ffT = p5.tile([128, FB, TCH], BF16, tag="ffT", bufs=1)
                    for f in range(FB):
                        wgf = p5.tile([128, DT, 256], BF16, tag="wgf", bufs=3)
                        nc.sync.dma_start(out=wgf, in_=wg_ext[f])
                        gps = ps5.tile([128, TCH], F32, tag="gps")
                        ups = ps5.tile([128, TCH], F32, tag="ups")
                        for kd in range(DT):
                            nc.tensor.matmul(gps, wgf[:, kd, 0:128],
                                             h2c[:, kd],
                                             start=kd == 0, stop=kd == DT - 1)
                        for kd in range(DT):
                            nc.tensor.matmul(ups, wgf[:, kd, 128:256],
                                             h2c[:, kd],
                                             start=kd == 0, stop=kd == DT - 1)
                        ga = p5.tile([128, TCH], F32, tag="ga")
                        nc.scalar.activation(out=ga, in_=gps,
                                             func=AF.Gelu_apprx_tanh)
                        nc.vector.tensor_tensor(ffT[:, f], ga, ups,
                                                op=ALU.mult)
                    for dch in range(DCH):
                        dsl = slice(dch * TCH, (dch + 1) * TCH)
                        wlc = p5.tile([128, FB, TCH], BF16, tag="wlc", bufs=2)
                        nc.sync.dma_start(out=wlc, in_=wl_ext[dch])
                        for tt in range(CHT):
                            dps = ps5.tile([128, TCH], F32, tag=f"dps{tt}",
                                           bufs=1)
                            for f in range(FB):
                                nc.tensor.matmul(
                                    dps, ffT[:, f, tt * 128:(tt + 1) * 128],
                                    wlc[:, f],
                                    start=f == 0, stop=f == FB - 1)
                            od = p5.tile([128, TCH], BF16, tag="od", bufs=3)
                            nc.scalar.copy(out=od, in_=dps)
                            nc.sync.dma_start(
                                out=down_d[r][tt * 128:(tt + 1) * 128, dsl],
                                in_=od)
                    nc.gpsimd.collective_compute(
                        "ReduceScatter", ALU.add, replica_groups=GROUPS,
                        ins=[down_d[r].opt()], outs=[down_r[r].opt()])
                    if r > 0:
                        final_block(r - 1)
                final_block(NCH - 1)
    nc.compile()
    return nc


# ---------------------------------------------------------------------------
# host side
# ---------------------------------------------------------------------------

def _pa(w, inner=128):
    """[A*128, N] -> [128, A, N] partition-major layout."""
    a = w.shape[0] // inner
    return np.ascontiguousarray(
        w.reshape(a, inner, w.shape[1]).transpose(1, 0, 2))


def make_in_maps(cfg, x, positions, attn_mask, scale_attn, w_q, w_kv, w_o,
                 scale_ffn, w_gating, w_linear):
    T, D, H, HEADS, FS = cfg["T"], cfg["D"], cfg["H"], cfg["HEADS"], cfg["FS"]
    SLICE = T // 4
    TCH = SLICE
    NCH = T // TCH
    BS = TCH // 4
    DT, FB, DCH = D // 128, FS // 128, D // TCH
    bf = ml_dtypes.bfloat16
    s1a = (1.0 + np.asarray(scale_attn, np.float32))[:, None]
    s1f = (1.0 + np.asarray(scale_ffn, np.float32))[:, None]
    k_w = (np.asarray(w_kv[0, 0], np.float32) * s1a)
    v_w = (np.asarray(w_kv[1, 0], np.float32) * s1a)
    wkv_h = _pa(np.concatenate([k_w, v_w], axis=1).astype(bf))
    freq = 10000.0 ** (2.0 / H * np.arange(H // 2, dtype=np.float32))
    in_maps = []
    for c in range(8):
        b, r = divmod(c, 4)
        hsel = slice(r * HEADS, (r + 1) * HEADS)
        wq_c = np.asarray(w_q[hsel], np.float32) * s1a[None] * H ** -0.5
        wq_c = _pa(np.concatenate(list(wq_c), axis=1).astype(bf))
        wo_c = _pa(np.concatenate(list(np.asarray(w_o[hsel], np.float32)),
                                  axis=0).astype(bf))
        fsel = slice(r * FS, (r + 1) * FS)
        # wg: [FB, 128, DT, 256] — per F-block, partition-major, gate|up cols
        gate = (np.asarray(w_gating[0][:, fsel], np.float32) * s1f).astype(bf)
        up = (np.asarray(w_gating[1][:, fsel], np.float32) * s1f).astype(bf)
        gate = gate.reshape(DT, 128, FB, 128).transpose(2, 1, 0, 3)
        up = up.reshape(DT, 128, FB, 128).transpose(2, 1, 0, 3)
        wg_c = np.ascontiguousarray(np.concatenate([gate, up], axis=3))
        # wl: [DCH, 128, FB, TCH]
        wl_c = np.asarray(w_linear[fsel], np.float32).astype(bf)
        wl_c = np.ascontiguousarray(
            wl_c.reshape(FB, 128, DCH, TCH).transpose(2, 1, 0, 3))
        pos = np.asarray(positions[b], np.float32)
        rad = pos[None, :] / freq[:, None]                       # [H/2, T]
        xb = np.ascontiguousarray(np.asarray(x[b], np.float32))
        xsl = np.concatenate([xb[ch * TCH + r * BS: ch * TCH + (r + 1) * BS]
                              for ch in range(NCH)], axis=0)
        in_maps.append({
            "x": xb,
            "x_slice": np.ascontiguousarray(xsl),
            "wq": wq_c, "wkv": wkv_h, "wo": wo_c, "wg": wg_c, "wl": wl_c,
            "sin": np.ascontiguousarray(np.sin(rad)),
            "cos": np.ascontiguousarray(np.cos(rad)),
        })
    return in_maps


def assemble(cfg, results, B):
    T, D = cfg["T"], cfg["D"]
    TCH = T // 4
    NCH = T // TCH
    BS = TCH // 4
    out = np.empty((B, T, D), np.float32)
    for c in range(8):
        b, r = divmod(c, 4)
        res = results[c]["out"]
        for ch in range(NCH):
            out[b, ch * TCH + r * BS: ch * TCH + (r + 1) * BS] = \
                res[ch * BS:(ch + 1) * BS]
    return out


# cached compiled program + jitted runner -----------------------------------

_CACHE = {}


def _get_runner(cfg_key, cfg):
    if cfg_key in _CACHE:
        return _CACHE[cfg_key]
    runner = _runner_from_nc(build(cfg))
    _CACHE[cfg_key] = runner
    return runner


def _runner_from_nc(nc):
    import jax
    from jax.experimental.shard_map import shard_map
    from jax.sharding import Mesh, PartitionSpec
    from concourse import bass2jax

    bass2jax.install_neuronx_cc_hook()

    partition_name = (nc.partition_id_tensor.name
                      if nc.partition_id_tensor else None)
    in_names, out_names, out_avals, zero_shapes = [], [], [], []
    for alloc in nc.m.functions[0].allocations:
        if not isinstance(alloc, mybir.MemoryLocationSet):
            continue
        name = alloc.memorylocations[0].name
        if alloc.kind == "ExternalInput":
            if name != partition_name:
                in_names.append(name)
        elif alloc.kind == "ExternalOutput":
            out_names.append(name)
            shape = tuple(alloc.tensor_shape)
            dtype = mybir.dt.np(alloc.dtype)
            out_avals.append(jax.core.ShapedArray(shape, dtype))
            zero_shapes.append((shape, dtype))
    n_params = len(in_names)
    all_in_names = in_names + out_names
    if partition_name is not None:
        all_in_names = all_in_names + [partition_name]

    def _body(*args):
        operands = list(args)
        if partition_name is not None:
            operands.append(bass2jax.partition_id_tensor())
        outs = bass2jax._bass_exec_p.bind(
            *operands,
            out_avals=tuple(out_avals),
            in_names=tuple(all_in_names),
            out_names=tuple(out_names),
            lowering_input_output_aliases=(),
            sim_require_finite=True,
            sim_require_nnan=True,
            nc=nc,
        )
        return tuple(outs)

    n_outs = len(out_names)
    donate = tuple(range(n_params, n_params + n_outs))
    devices = jax.devices()[:8]
    mesh = Mesh(np.asarray(devices), ("core",))
    in_specs = (PartitionSpec("core"),) * (n_params + n_outs)
    out_specs = (PartitionSpec("core"),) * n_outs
    sharded = jax.jit(
        shard_map(_body, mesh=mesh, in_specs=in_specs, out_specs=out_specs,
                  check_rep=False),
        donate_argnums=donate, keep_unused=True)

    class Runner:
        pass

    runner = Runner()
    runner.sharded = sharded
    runner.mesh = mesh
    runner.in_names = in_names
    runner.out_names = out_names
    runner.out_avals = out_avals
    runner.zero_shapes = zero_shapes

    def concat_inputs(in_maps):
        return [np.concatenate([np.asarray(m[name]) for m in in_maps],
                               axis=0) for name in in_names]

    def make_zeros():
        return [np.zeros((8 * s[0], *s[1:]), d) for s, d in zero_shapes]

    def split_outputs(out_arrs):
        return [
            {name: np.asarray(out_arrs[i]).reshape(8, *out_avals[i].shape)[c]
             for i, name in enumerate(out_names)}
            for c in range(8)
        ]

    runner.concat_inputs = concat_inputs
    runner.make_zeros = make_zeros
    runner.split_outputs = split_outputs

    def run(in_maps):
        out_arrs = sharded(*concat_inputs(in_maps), *make_zeros())
        return split_outputs(out_arrs)

    runner.run = run
    return runner


def run_cfg(cfg, inputs):
    cfg_key = tuple(sorted(cfg.items()))
    runner = _get_runner(cfg_key, cfg)
    in_maps = make_in_maps(cfg, **inputs)
    results = runner.run(in_maps)
    return assemble(cfg, results, np.asarray(inputs["x"]).shape[0])


def kernel(**inputs):
    return run_cfg(FULL_CFG, inputs)

